# revision 41
# baseline (speedup 1.0000x reference)
"""Cross-attention kernel for TRN2, 8 NeuronCores, data-parallel over points.

Math (derived from the reference):
  [qk | qp][n] = q[n] @ [MA | MQ]     MA = (Wq.T Wk)*s, MQ = Wq.T
  scores[n,w]  = qk[n] . k[w,n]
  attn[n]      = softmax_w(scores[n])
  vmixT        = sum_w v_w.T-weighted: vmixT[:,n] = sum_w attn[n,w] * v[w,n,:]
  y[n]         = gelu(vmixT.T @ MB + bo) + qp[n],  MB = Wv.T Wo.T
  out[c][8*i+j] = y[c*4096+i]   (row replication done on host)

Engine split per 128-point tile:
  PE:     q projections (host pre-transposes q, so no PE transposes),
          vmixT via diag(attn_w)-moving matmuls (v is the fp8 stationary),
          y = vmixT.T @ MB + bias matmul.
  DVE:    scores broadcast-mult (bf16 2x), fold cascade + final
          tensor_reduce, softmax denominator reduce + reciprocal + e*rs
          fold + diag build per PAIR (one 2048-el 1x TT beats two STTs on
          per-op overhead; per-TILE for the kernel's final pair to shorten
          the tail), residual add.
  ACT:    one merged [qk|qp] PSUM->SBUF cast, vmixT cast, Exp (NO
          accum_out: each accumulator read is a separate 280ns ACT
          instruction on the reciprocal's critical path), Gelu per group.

Scheduling notes (all measured on HW):
  - v tiles prefetched at group top (-3us: the vmix otherwise waits).
  - gelu(g-1) emitted in group g AFTER its exps: the Gelu<->Exp table
    loads (1283ns each) otherwise precede the exps the softmax chain
    waits on.
  - qT DMA half a group before its projections; projections + casts
    emitted after the exps. PE runs them mid-group with data resident.
  - per-pair softmax chains stay INLINE: deferring them delays pair0's
    diag and cancels the stall win.
Dead ends (measured): strided PE moving operands are ~3x slower (any
m-major diag layout dies); K=64/tile_position matmuls crash the HW path;
GpSimd compute inflates concurrent DVE ops ~33% (SBUF port contention);
tensor_scalar 4x mode loses to per-op overhead (245ns/op) at [P,128]
sizes; host-precomputed qp costs more in DMA than it saves on ACT.
GpSimd is UNUSED; DVE active ~125-127us is the wall (score mult+folds
~84us irreducible at 2 elem/cy/partition, diag ~35us at 1x).
DMA traffic per core: k bf16 16MB, v fp8 8MB, q bf16 2MB, out bf16 2MB.
"""

import ml_dtypes
import numpy as np

import concourse.bass as bass
import concourse.mybir as mybir
import concourse.tile as tile
from concourse import bacc
from concourse.bass_utils import run_bass_kernel_spmd

N_CORES = 8
N_TOTAL = 32768
NC_PTS = N_TOTAL // N_CORES  # 4096 points per core
D = 256
V = 8
P = 128
G = 4  # tiles per group (gelu batching + q/out DMA batching)
N_TILES = NC_PTS // P  # 32
F32 = mybir.dt.float32
BF16 = mybir.dt.bfloat16
FP8 = mybir.dt.float8e4
NP_BF16 = ml_dtypes.bfloat16
NP_FP8 = ml_dtypes.float8_e4m3
AX = mybir.AxisListType
OP = mybir.AluOpType
AF = mybir.ActivationFunctionType


def _bcast(ap, axis_count, after_dims):
    """Insert a [0, axis_count] broadcast dim before the last `after_dims`
    dims of `ap`'s access pattern."""
    dims = list(ap.ap)
    pos = len(dims) - after_dims
    dims = dims[:pos] + [[0, axis_count]] + dims[pos:]
    return bass.AP(tensor=ap.tensor, offset=ap.offset, ap=dims)


def _pairview(ap, lo, n):
    """[P, 2, V, D] k-pair tile -> 3D [P, 2*V, n] view at column offset lo.
    Legal because stride(s) == V*stride(v); keeping the AP 3-dim preserves
    the DVE 2x perf mode (4-dim APs were measured to drop it)."""
    d = list(ap.ap)
    assert len(d) == 4 and d[1][0] == d[2][1] * d[2][0]
    return bass.AP(
        tensor=ap.tensor,
        offset=ap.offset + lo,
        ap=[d[0], [d[2][0], d[1][1] * d[2][1]], [d[3][0], n]],
    )


def build_bass(n_tiles: int = N_TILES, gelu: bool = True):
    nc = bacc.Bacc(
        "TRN2", target_bir_lowering=False, debug=False, num_devices=N_CORES
    )
    assert n_tiles % G == 0
    n_groups = n_tiles // G
    npts = n_tiles * P
    # q pre-transposed on host: [d-half(part), 2, npts]
    q_d = nc.dram_tensor("qT", [P, 2, npts], BF16, kind="ExternalInput")
    k_d = nc.dram_tensor("k", [npts, V, D], BF16, kind="ExternalInput")
    v_d = nc.dram_tensor("v", [npts, V, D], FP8, kind="ExternalInput")
    mamq_d = nc.dram_tensor("mamq", [2, P, 2 * D], BF16, kind="ExternalInput")
    mb_d = nc.dram_tensor("mb", [2, P, D], BF16, kind="ExternalInput")
    bo_d = nc.dram_tensor("bo_r", [1, D], BF16, kind="ExternalInput")
    ones_d = nc.dram_tensor("ones_r", [1, P], BF16, kind="ExternalInput")
    idp_d = nc.dram_tensor("idp", [P, P], BF16, kind="ExternalInput")
    # unique rows only, packed [P, n_tiles, D] bf16; host replicates 8x
    out_d = nc.dram_tensor("out", [P, n_tiles, D], BF16, kind="ExternalOutput")

    with tile.TileContext(nc) as tc:
        with (
            tc.tile_pool(name="singles", bufs=1) as singles,
            tc.tile_pool(name="qio", bufs=2) as qio,
            tc.tile_pool(name="io", bufs=6) as io,
            tc.tile_pool(name="work", bufs=4) as work,
            tc.tile_pool(name="tl", bufs=6) as tl,
            tc.tile_pool(name="gwork", bufs=2) as gwork,
            tc.tile_pool(name="ps", bufs=2, space="PSUM") as ps,
            tc.tile_pool(name="psy", bufs=2, space="PSUM") as psy,
        ):
            mamq_t = singles.tile([P, 2, 2 * D], BF16)
            mb_t = singles.tile([P, 2, D], BF16)
            bo_t = singles.tile([1, D], BF16)
            ones_t = singles.tile([1, P], BF16)
            idp_t = singles.tile([P, P], BF16)
            nc.sync.dma_start(
                out=mamq_t, in_=mamq_d.ap().rearrange("h p d -> p h d")
            )

            def emit_q_dma(gq):
                """qT DMA for group gq, issued half a group before its
                projections so the PE never waits on it."""
                gq0 = gq * G
                qT_g = qio.tile([P, 2, G * P], BF16, tag="q")
                nc.sync.dma_start(
                    out=qT_g, in_=q_d.ap()[:, :, gq0 * P : (gq0 + G) * P]
                )
                return qT_g

            def emit_qkqp_mm(qT_g):
                """[qk|qp] projections + casts. Emitted AFTER the current
                group's exps: the casts then never sit in front of the exps
                on the ACT queue (they were stalling every reciprocal), and
                the qT DMA from half a group ago is long since landed."""
                bs = []
                for t in range(G):
                    if t % 2 == 0:
                        qkqp_b2 = tl.tile([P, 2, 2 * D], BF16, tag="qkqp_b")
                        bs.append(qkqp_b2)
                    qkqp_ps = ps.tile([P, 2 * D], F32, tag="qkqp")
                    nc.tensor.matmul(
                        qkqp_ps,
                        qT_g[:, 0, t * P : (t + 1) * P],
                        mamq_t[:, 0],
                        start=True,
                        stop=False,
                    )
                    nc.tensor.matmul(
                        qkqp_ps,
                        qT_g[:, 1, t * P : (t + 1) * P],
                        mamq_t[:, 1],
                        start=False,
                        stop=True,
                    )
                    nc.scalar.copy(bs[t // 2][:, t % 2], qkqp_ps)
                return bs

            def emit_k_group(gq):
                """Two slice-DMAs per pair tile: the first score mult then
                waits on 512KB, not the whole 1MB pair (the SDMA engines
                round-robin all queues, so everything in flight finishes
                together — smaller completion units start compute sooner)."""
                ks = []
                for half in range(G // 2):
                    gq0 = gq * G
                    k2_t = io.tile([P, 2, V, D], BF16, tag="k")
                    for u in range(2):
                        ti = gq0 + 2 * half + u
                        nc.sync.dma_start(
                            out=k2_t[:, u],
                            in_=k_d.ap()[ti * P : (ti + 1) * P],
                        )
                    ks.append(k2_t)
                return ks

            # startup-critical first: the q-projection chain (mamq is
            # already in flight), then group 0's k tiles; the y-stage
            # constants can land much later
            cur_bs = emit_qkqp_mm(emit_q_dma(0))
            k2_first = emit_k_group(0)
            nc.sync.dma_start(
                out=mb_t, in_=mb_d.ap().rearrange("h p d -> p h d")
            )
            nc.sync.dma_start(out=bo_t, in_=bo_d.ap())
            nc.sync.dma_start(out=ones_t, in_=ones_d.ap())
            nc.sync.dma_start(out=idp_t, in_=idp_d.ap())
            pending = None

            for gi in range(n_groups):
                g0 = gi * G
                y_ps = psy.tile([P, G, D], F32, tag="y")
                qkqp_bs = cur_bs
                sm_g = work.tile([P, G], F32, tag="smg")
                e_g = work.tile([P, G, V], F32, tag="eg")
                diag_ss = []
                chain_q = []

                k2_ts = k2_first if gi == 0 else emit_k_group(gi)

                def emit_v_group():
                    vts = []
                    for s in range(G // 2):
                        ti = g0 + 2 * s
                        sl2 = slice(ti * P, (ti + 2) * P)
                        v2_t = io.tile([P, 2, V, D], FP8, tag="v")
                        nc.sync.dma_start(
                            out=v2_t,
                            in_=v_d.ap()[sl2].rearrange(
                                "(s p) v d -> p s v d", p=P
                            ),
                        )
                        vts.append(v2_t)
                    return vts

                # v prefetched at group top: the pair DMA lands well before
                # its vmix instead of being issued right when it's needed.
                # EXCEPT group 0: its v would round-robin with the first k
                # tiles and push the first score mult out by ~5us.
                v2_ts = emit_v_group() if gi > 0 else None

                qT_next = emit_q_dma(gi + 1) if gi + 1 < n_groups else None

                for s in range(G // 2):
                    k2 = k2_ts[s]

                    # bcast mults stay per-tile (a 4-dim paired AP drops the
                    # 2x mode); folds/reduce run per PAIR via 3D merged views
                    for u in range(2):
                        nc.vector.tensor_tensor(
                            k2[:, u],
                            _bcast(qkqp_bs[s][:, u, 0:D], V, 1),
                            k2[:, u],
                            op=OP.mult,
                        )
                    k2ap = k2[:, 0:2, 0:V, 0:D]
                    nc.vector.tensor_tensor(
                        _pairview(k2ap, 0, 128),
                        _pairview(k2ap, 0, 128),
                        _pairview(k2ap, 128, 128),
                        op=OP.add,
                    )
                    nc.vector.tensor_tensor(
                        _pairview(k2ap, 0, 64),
                        _pairview(k2ap, 0, 64),
                        _pairview(k2ap, 64, 64),
                        op=OP.add,
                    )
                    nc.vector.tensor_tensor(
                        _pairview(k2ap, 0, 32),
                        _pairview(k2ap, 0, 32),
                        _pairview(k2ap, 32, 32),
                        op=OP.add,
                    )
                    scores2 = work.tile([P, 2, V], F32, tag="scores")
                    nc.vector.tensor_reduce(
                        scores2, _pairview(k2ap, 0, 32), axis=AX.X, op=OP.add
                    )

                    # softmax (no max-shift: scores ~N(0,1), f32 exp safe);
                    # NO accum_out: each one costs a separate 280ns ACT
                    # accumulator-read that sits on the reciprocal's critical
                    # path -- a tiny DVE reduce is cheaper end-to-end
                    for u in range(2):
                        t = 2 * s + u
                        nc.scalar.activation(
                            e_g[:, t],
                            scores2[:, u],
                            AF.Exp,
                        )

                    # per-PAIR sum + recip + rs-fold + diag: the pair's vmix
                    # can then start two exps earlier than with a group-wide
                    # reciprocal (shorter tail, same DVE total: one 2048-el
                    # 1x TT beats two 1024-el STTs on per-op overhead)
                    # last pair of the kernel: per-TILE softmax tail so the
                    # final vmix chain starts ~1.2us earlier; everywhere else
                    # per-PAIR (fewer ops). For non-last groups the whole
                    # chain is DEFERRED until after the other pair's score
                    # ops so the sm reduce never waits on the ACT exps.
                    def emit_chain(s=s, tile_split=(gi == n_groups - 1) and (s == G // 2 - 1)):
                        subs = ((0,), (1,)) if tile_split else ((0, 1),)
                        diag_s = work.tile([P, 2 * V, P], BF16, tag="diag")
                        for sub in subs:
                            u0, un = sub[0], len(sub)
                            t0c = 2 * s + u0
                            nc.vector.tensor_reduce(
                                sm_g[:, t0c : t0c + un],
                                e_g[:, t0c : t0c + un, 0:V],
                                axis=AX.X,
                                op=OP.add,
                            )
                            rs_s = work.tile([P, un], F32, tag="rsg")
                            nc.vector.reciprocal(rs_s, sm_g[:, t0c : t0c + un])
                            E_s = work.tile([P, un, V], F32, tag="Eg")
                            rs_ap = rs_s[:, 0:un]
                            nc.vector.tensor_tensor(
                                E_s,
                                e_g[:, t0c : t0c + un],
                                bass.AP(
                                    tensor=rs_ap.tensor,
                                    offset=rs_ap.offset,
                                    ap=[list(rs_ap.ap[0]), list(rs_ap.ap[1]), [0, V]],
                                ),
                                op=OP.mult,
                            )
                            Es_ap = E_s[:, 0:un, 0:V]
                            dEs = list(Es_ap.ap)
                            idp_ap = idp_t[:, 0:P]
                            nc.vector.tensor_tensor(
                                diag_s[:, u0 * V : (u0 + un) * V],
                                bass.AP(
                                    tensor=idp_ap.tensor,
                                    offset=idp_ap.offset,
                                    ap=[list(idp_ap.ap[0]), [0, un * V], [1, P]],
                                ),
                                bass.AP(
                                    tensor=Es_ap.tensor,
                                    offset=Es_ap.offset,
                                    ap=[dEs[0], [dEs[2][0], un * V], [0, P]],
                                ),
                                op=OP.mult,
                            )
                        diag_ss.append(diag_s)

                    emit_chain()


                if v2_ts is None:
                    v2_ts = emit_v_group()

                # next group's projections (qT landed half a group ago)
                next_bs = emit_qkqp_mm(qT_next) if qT_next is not None else None

                # previous group's gelu FIRST NOW (emitted here, after this
                # group's exps, so its Gelu<->Exp table loads never precede
                # the exps the reciprocal is waiting on), then its residual
                if pending is not None:
                    p_ps, p_bs, p_g0 = pending
                    p_gl = gwork.tile([P, G, D], BF16, tag="gl")
                    nc.scalar.activation(
                        p_gl, p_ps, AF.Gelu if gelu else AF.Identity
                    )
                    y_out = gwork.tile([P, G, D], BF16, tag="yout")
                    for s2 in range(G // 2):
                        nc.vector.tensor_tensor(
                            y_out[:, 2 * s2 : 2 * s2 + 2],
                            p_gl[:, 2 * s2 : 2 * s2 + 2],
                            p_bs[s2][:, :, D : 2 * D],
                            op=OP.add,
                        )
                    nc.scalar.dma_start(
                        out=out_d.ap()[:, p_g0 : p_g0 + G], in_=y_out
                    )

                for t in range(G):
                    ti = g0 + t
                    v_t = v2_ts[t // 2][:, t % 2]

                    # vmixT[d,n] = sum_w v_w[n,d]*attn_w[n]: stat = v (fp8)
                    vmixT_ps = ps.tile([P, 2, P], F32, tag="vmixT")
                    for h in range(2):
                        for w in range(V):
                            nc.tensor.matmul(
                                vmixT_ps[:, h],
                                v_t[:, w, h * P : (h + 1) * P],
                                diag_ss[t // 2][:, (t % 2) * V + w],
                                start=(w == 0),
                                stop=(w == V - 1),
                            )
                    vT_b = work.tile([P, 2, P], BF16, tag="vT")
                    nc.scalar.copy(vT_b, vmixT_ps)

                    # y = vmixT.T @ MB + bo (bias as a K=1 matmul)
                    nc.tensor.matmul(
                        y_ps[:, t], ones_t, bo_t, start=True, stop=False
                    )
                    nc.tensor.matmul(
                        y_ps[:, t], vT_b[:, 0], mb_t[:, 0], start=False, stop=False
                    )
                    nc.tensor.matmul(
                        y_ps[:, t], vT_b[:, 1], mb_t[:, 1], start=False, stop=True
                    )

                    # LAST group: drain the epilogue per pair so the final
                    # gelu/add/store pipeline with the remaining vmix work
                    # instead of serializing after it (trims the kernel tail)
                    if gi == n_groups - 1 and (t % 2 == 1 or t == G - 2):
                        if t == G - 2 or t == G - 1:
                            lo, n_t = t, 1
                        else:
                            lo, n_t = t - 1, 2
                        gl_h = gwork.tile([P, n_t, D], BF16, tag="glh")
                        nc.scalar.activation(
                            gl_h,
                            y_ps[:, lo : lo + n_t],
                            AF.Gelu if gelu else AF.Identity,
                        )
                        yo_h = gwork.tile([P, n_t, D], BF16, tag="yoh")
                        nc.vector.tensor_tensor(
                            yo_h,
                            gl_h,
                            qkqp_bs[lo // 2][
                                :, lo % 2 : lo % 2 + n_t, D : 2 * D
                            ],
                            op=OP.add,
                        )
                        nc.scalar.dma_start(
                            out=out_d.ap()[:, g0 + lo : g0 + lo + n_t],
                            in_=yo_h,
                        )

                if gi == n_groups - 1:
                    continue
                # gelu deferred: emitted in the next group after its exps
                pending = (y_ps, qkqp_bs, g0)
                cur_bs = next_bs

    nc.compile()
    return nc


_NC_CACHE = {}


def _get_nc(n_tiles: int = N_TILES):
    if n_tiles not in _NC_CACHE:
        _NC_CACHE[n_tiles] = build_bass(n_tiles)
    return _NC_CACHE[n_tiles]


def _host_prep(Wq, Wk, Wv, Wo, bo):
    Wq = np.asarray(Wq, dtype=np.float32)
    Wk = np.asarray(Wk, dtype=np.float32)
    Wv = np.asarray(Wv, dtype=np.float32)
    Wo = np.asarray(Wo, dtype=np.float32)
    bo = np.asarray(bo, dtype=np.float32)
    scale = np.float32(1.0) / np.sqrt(np.float32(D))
    ma = (Wq.T @ Wk) * scale
    mq = Wq.T
    mamq = (
        np.concatenate([ma, mq], axis=1).reshape(2, P, 2 * D).astype(NP_BF16)
    )
    mb = (Wv.T @ Wo.T).reshape(2, P, D).astype(NP_BF16)
    bo_r = bo.reshape(1, D).astype(NP_BF16)
    ones_r = np.ones((1, P), dtype=NP_BF16)
    idp = np.eye(P, dtype=np.float32).astype(NP_BF16)
    return (
        np.ascontiguousarray(mamq),
        np.ascontiguousarray(mb),
        bo_r,
        ones_r,
        idp,
    )


def make_in_maps(q, k, v, Wq, Wk, Wv, Wo, bo):
    q = np.asarray(q, dtype=np.float32)
    k = np.asarray(k, dtype=np.float32)
    v = np.asarray(v, dtype=np.float32)
    mamq, mb, bo_r, ones_r, idp = _host_prep(Wq, Wk, Wv, Wo, bo)
    in_maps = []
    for c in range(N_CORES):
        sl = slice(c * NC_PTS, (c + 1) * NC_PTS)
        # qT: [128(d-half part), 2, npts]
        q_c = np.ascontiguousarray(
            q[0, sl].T.reshape(2, P, NC_PTS).transpose(1, 0, 2)
        ).astype(NP_BF16)
        k_c = k[:, sl].transpose(1, 0, 2).astype(NP_BF16)
        v_c = v[:, sl].transpose(1, 0, 2).astype(NP_FP8)
        in_maps.append(
            {
                "qT": q_c,
                "k": np.ascontiguousarray(k_c),
                "v": np.ascontiguousarray(v_c),
                "mamq": mamq,
                "mb": mb,
                "bo_r": bo_r,
                "ones_r": ones_r,
                "idp": np.ascontiguousarray(idp),
            }
        )
    return in_maps


def gather_out(results):
    """[P, N_TILES, D] bf16 per core -> [8, 32768, 256] f32, 8x replicated."""
    out = np.empty((N_CORES, N_TOTAL, D), dtype=np.float32)
    for c in range(N_CORES):
        y = (
            results[c]["out"]
            .astype(np.float32)
            .transpose(1, 0, 2)
            .reshape(NC_PTS, D)
        )
        out[c] = np.repeat(y, V, axis=0)
    return out


def kernel(q, k, v, Wq, Wk, Wv, Wo, bo):
    nc = _get_nc()
    in_maps = make_in_maps(q, k, v, Wq, Wk, Wv, Wo, bo)
    res = run_bass_kernel_spmd(nc, in_maps, core_ids=list(range(N_CORES)))
    return gather_out(res.results)



# revision 42
# speedup vs baseline: 1.0277x; 1.0277x over previous
"""Cross-attention kernel for TRN2, 8 NeuronCores, data-parallel over points.

Math (derived from the reference):
  [qk | qp][n] = q[n] @ [MA | MQ]     MA = (Wq.T Wk)*s, MQ = Wq.T
  scores[n,w]  = qk[n] . k[w,n]
  attn[n]      = softmax_w(scores[n])
  vmixT        = sum_w v_w.T-weighted: vmixT[:,n] = sum_w attn[n,w] * v[w,n,:]
  y[n]         = gelu(vmixT.T @ MB + bo) + qp[n],  MB = Wv.T Wo.T
  out[c][8*i+j] = y[c*4096+i]   (row replication done on host)

Engine split per 128-point tile:
  PE:     q projections (host pre-transposes q, so no PE transposes),
          vmixT via diag(attn_w)-moving matmuls (v is the fp8 stationary),
          y = vmixT.T @ MB + bias matmul.
  DVE:    scores broadcast-mult (bf16 2x), fold cascade + final
          tensor_reduce, softmax denominator reduce + reciprocal + e*rs
          fold + diag build per PAIR (one 2048-el 1x TT beats two STTs on
          per-op overhead; per-TILE for the kernel's final pair to shorten
          the tail), residual add.
  ACT:    one merged [qk|qp] PSUM->SBUF cast, vmixT cast, Exp (NO
          accum_out: each accumulator read is a separate 280ns ACT
          instruction on the reciprocal's critical path), Gelu per group.

Scheduling notes (all measured on HW):
  - v tiles prefetched at group top (-3us: the vmix otherwise waits).
  - gelu(g-1) emitted in group g AFTER its exps: the Gelu<->Exp table
    loads (1283ns each) otherwise precede the exps the softmax chain
    waits on.
  - qT DMA half a group before its projections; projections + casts
    emitted after the exps. PE runs them mid-group with data resident.
  - per-pair softmax chains stay INLINE: deferring them delays pair0's
    diag and cancels the stall win.
Dead ends (measured): strided PE moving operands are ~3x slower (any
m-major diag layout dies); K=64/tile_position matmuls crash the HW path;
GpSimd compute inflates concurrent DVE ops ~33% (SBUF port contention);
tensor_scalar 4x mode loses to per-op overhead (245ns/op) at [P,128]
sizes; host-precomputed qp costs more in DMA than it saves on ACT.
GpSimd is UNUSED; DVE active ~125-127us is the wall (score mult+folds
~84us irreducible at 2 elem/cy/partition, diag ~35us at 1x).
DMA traffic per core: k bf16 16MB, v fp8 8MB, q bf16 2MB, out bf16 2MB.
"""

import ml_dtypes
import numpy as np

import concourse.bass as bass
import concourse.mybir as mybir
import concourse.tile as tile
from concourse import bacc
from concourse.bass_utils import run_bass_kernel_spmd

N_CORES = 8
N_TOTAL = 32768
NC_PTS = N_TOTAL // N_CORES  # 4096 points per core
D = 256
V = 8
P = 128
G = 4  # tiles per group (gelu batching + q/out DMA batching)
N_TILES = NC_PTS // P  # 32
F32 = mybir.dt.float32
BF16 = mybir.dt.bfloat16
FP8 = mybir.dt.float8e4
NP_BF16 = ml_dtypes.bfloat16
NP_FP8 = ml_dtypes.float8_e4m3
AX = mybir.AxisListType
OP = mybir.AluOpType
AF = mybir.ActivationFunctionType


def _bcast(ap, axis_count, after_dims):
    """Insert a [0, axis_count] broadcast dim before the last `after_dims`
    dims of `ap`'s access pattern."""
    dims = list(ap.ap)
    pos = len(dims) - after_dims
    dims = dims[:pos] + [[0, axis_count]] + dims[pos:]
    return bass.AP(tensor=ap.tensor, offset=ap.offset, ap=dims)


def _pairview(ap, lo, n):
    """[P, 2, V, D] k-pair tile -> 3D [P, 2*V, n] view at column offset lo.
    Legal because stride(s) == V*stride(v); keeping the AP 3-dim preserves
    the DVE 2x perf mode (4-dim APs were measured to drop it)."""
    d = list(ap.ap)
    assert len(d) == 4 and d[1][0] == d[2][1] * d[2][0]
    return bass.AP(
        tensor=ap.tensor,
        offset=ap.offset + lo,
        ap=[d[0], [d[2][0], d[1][1] * d[2][1]], [d[3][0], n]],
    )


def build_bass(n_tiles: int = N_TILES, gelu: bool = True):
    nc = bacc.Bacc(
        "TRN2", target_bir_lowering=False, debug=False, num_devices=N_CORES
    )
    assert n_tiles % G == 0
    n_groups = n_tiles // G
    npts = n_tiles * P
    # q pre-transposed on host: [d-half(part), 2, npts]
    q_d = nc.dram_tensor("qT", [P, 2, npts], BF16, kind="ExternalInput")
    k_d = nc.dram_tensor("k", [npts, V, D], BF16, kind="ExternalInput")
    v_d = nc.dram_tensor("v", [npts, V, D], FP8, kind="ExternalInput")
    mamq_d = nc.dram_tensor("mamq", [2, P, 2 * D], BF16, kind="ExternalInput")
    mb_d = nc.dram_tensor("mb", [2, P, D], BF16, kind="ExternalInput")
    bo_d = nc.dram_tensor("bo_r", [1, D], BF16, kind="ExternalInput")
    ones_d = nc.dram_tensor("ones_r", [1, P], BF16, kind="ExternalInput")
    idp_d = nc.dram_tensor("idp", [P, P], BF16, kind="ExternalInput")
    # unique rows only, packed [P, n_tiles, D] bf16; host replicates 8x
    out_d = nc.dram_tensor("out", [P, n_tiles, D], BF16, kind="ExternalOutput")

    with tile.TileContext(nc) as tc:
        with (
            tc.tile_pool(name="singles", bufs=1) as singles,
            tc.tile_pool(name="qio", bufs=2) as qio,
            tc.tile_pool(name="io", bufs=6) as io,
            tc.tile_pool(name="work", bufs=4) as work,
            tc.tile_pool(name="tl", bufs=6) as tl,
            tc.tile_pool(name="gwork", bufs=2) as gwork,
            tc.tile_pool(name="ps", bufs=2, space="PSUM") as ps,
            tc.tile_pool(name="psy", bufs=2, space="PSUM") as psy,
        ):
            mamq_t = singles.tile([P, 2, 2 * D], BF16)
            mb_t = singles.tile([P, 2, D], BF16)
            bo_t = singles.tile([1, D], BF16)
            ones_t = singles.tile([1, P], BF16)
            idp_t = singles.tile([P, P], BF16)
            nc.sync.dma_start(
                out=mamq_t, in_=mamq_d.ap().rearrange("h p d -> p h d")
            )

            def emit_q_dma(gq):
                """qT DMA for group gq, issued half a group before its
                projections so the PE never waits on it."""
                gq0 = gq * G
                qT_g = qio.tile([P, 2, G * P], BF16, tag="q")
                nc.sync.dma_start(
                    out=qT_g, in_=q_d.ap()[:, :, gq0 * P : (gq0 + G) * P]
                )
                return qT_g

            def emit_qkqp_mm(qT_g):
                """[qk|qp] projections + casts. Emitted AFTER the current
                group's exps: the casts then never sit in front of the exps
                on the ACT queue (they were stalling every reciprocal), and
                the qT DMA from half a group ago is long since landed."""
                bs = []
                for t in range(G):
                    if t % 2 == 0:
                        qkqp_b2 = tl.tile([P, 2, 2 * D], BF16, tag="qkqp_b")
                        bs.append(qkqp_b2)
                    qkqp_ps = ps.tile([P, 2 * D], F32, tag="qkqp")
                    nc.tensor.matmul(
                        qkqp_ps,
                        qT_g[:, 0, t * P : (t + 1) * P],
                        mamq_t[:, 0],
                        start=True,
                        stop=False,
                    )
                    nc.tensor.matmul(
                        qkqp_ps,
                        qT_g[:, 1, t * P : (t + 1) * P],
                        mamq_t[:, 1],
                        start=False,
                        stop=True,
                    )
                    nc.scalar.copy(bs[t // 2][:, t % 2], qkqp_ps)
                return bs

            def emit_k_group(gq):
                """Two slice-DMAs per pair tile: the first score mult then
                waits on 512KB, not the whole 1MB pair (the SDMA engines
                round-robin all queues, so everything in flight finishes
                together — smaller completion units start compute sooner)."""
                ks = []
                for half in range(G // 2):
                    gq0 = gq * G
                    k2_t = io.tile([P, 2, V, D], BF16, tag="k")
                    for u in range(2):
                        ti = gq0 + 2 * half + u
                        nc.sync.dma_start(
                            out=k2_t[:, u],
                            in_=k_d.ap()[ti * P : (ti + 1) * P],
                        )
                    ks.append(k2_t)
                return ks

            # startup-critical first: the q-projection chain (mamq is
            # already in flight), then group 0's k tiles; the y-stage
            # constants can land much later
            cur_bs = emit_qkqp_mm(emit_q_dma(0))
            k2_first = emit_k_group(0)
            nc.sync.dma_start(
                out=mb_t, in_=mb_d.ap().rearrange("h p d -> p h d")
            )
            nc.sync.dma_start(out=bo_t, in_=bo_d.ap())
            nc.sync.dma_start(out=ones_t, in_=ones_d.ap())
            nc.sync.dma_start(out=idp_t, in_=idp_d.ap())
            pending = None

            for gi in range(n_groups):
                g0 = gi * G
                y_ps = psy.tile([P, G, D], F32, tag="y")
                qkqp_bs = cur_bs
                sm_g = work.tile([P, G], F32, tag="smg")
                e_g = work.tile([P, G, V], F32, tag="eg")
                diag_ss = []
                chain_q = []

                k2_ts = k2_first if gi == 0 else emit_k_group(gi)

                def emit_v_group():
                    vts = []
                    for s in range(G // 2):
                        ti = g0 + 2 * s
                        sl2 = slice(ti * P, (ti + 2) * P)
                        v2_t = io.tile([P, 2, V, D], FP8, tag="v")
                        nc.sync.dma_start(
                            out=v2_t,
                            in_=v_d.ap()[sl2].rearrange(
                                "(s p) v d -> p s v d", p=P
                            ),
                        )
                        vts.append(v2_t)
                    return vts

                # v prefetched at group top: the pair DMA lands well before
                # its vmix instead of being issued right when it's needed
                v2_ts = emit_v_group()

                qT_next = emit_q_dma(gi + 1) if gi + 1 < n_groups else None

                for s in range(G // 2):
                    k2 = k2_ts[s]

                    # bcast mults stay per-tile (a 4-dim paired AP drops the
                    # 2x mode); folds/reduce run per PAIR via 3D merged views
                    for u in range(2):
                        nc.vector.tensor_tensor(
                            k2[:, u],
                            _bcast(qkqp_bs[s][:, u, 0:D], V, 1),
                            k2[:, u],
                            op=OP.mult,
                        )
                    k2ap = k2[:, 0:2, 0:V, 0:D]
                    nc.vector.tensor_tensor(
                        _pairview(k2ap, 0, 128),
                        _pairview(k2ap, 0, 128),
                        _pairview(k2ap, 128, 128),
                        op=OP.add,
                    )
                    nc.vector.tensor_tensor(
                        _pairview(k2ap, 0, 64),
                        _pairview(k2ap, 0, 64),
                        _pairview(k2ap, 64, 64),
                        op=OP.add,
                    )
                    nc.vector.tensor_tensor(
                        _pairview(k2ap, 0, 32),
                        _pairview(k2ap, 0, 32),
                        _pairview(k2ap, 32, 32),
                        op=OP.add,
                    )
                    scores2 = work.tile([P, 2, V], F32, tag="scores")
                    nc.vector.tensor_reduce(
                        scores2, _pairview(k2ap, 0, 32), axis=AX.X, op=OP.add
                    )

                    # softmax (no max-shift: scores ~N(0,1), f32 exp safe);
                    # NO accum_out: each one costs a separate 280ns ACT
                    # accumulator-read that sits on the reciprocal's critical
                    # path -- a tiny DVE reduce is cheaper end-to-end
                    for u in range(2):
                        t = 2 * s + u
                        nc.scalar.activation(
                            e_g[:, t],
                            scores2[:, u],
                            AF.Exp,
                        )

                    # per-PAIR sum + recip + rs-fold + diag: the pair's vmix
                    # can then start two exps earlier than with a group-wide
                    # reciprocal (shorter tail, same DVE total: one 2048-el
                    # 1x TT beats two 1024-el STTs on per-op overhead)
                    # last pair of the kernel: per-TILE softmax tail so the
                    # final vmix chain starts ~1.2us earlier; everywhere else
                    # per-PAIR (fewer ops). For non-last groups the whole
                    # chain is DEFERRED until after the other pair's score
                    # ops so the sm reduce never waits on the ACT exps.
                    def emit_chain(s=s, tile_split=(gi == n_groups - 1) and (s == G // 2 - 1)):
                        subs = ((0,), (1,)) if tile_split else ((0, 1),)
                        diag_s = work.tile([P, 2 * V, P], BF16, tag="diag")
                        for sub in subs:
                            u0, un = sub[0], len(sub)
                            t0c = 2 * s + u0
                            nc.vector.tensor_reduce(
                                sm_g[:, t0c : t0c + un],
                                e_g[:, t0c : t0c + un, 0:V],
                                axis=AX.X,
                                op=OP.add,
                            )
                            rs_s = work.tile([P, un], F32, tag="rsg")
                            nc.vector.reciprocal(rs_s, sm_g[:, t0c : t0c + un])
                            E_s = work.tile([P, un, V], F32, tag="Eg")
                            rs_ap = rs_s[:, 0:un]
                            nc.vector.tensor_tensor(
                                E_s,
                                e_g[:, t0c : t0c + un],
                                bass.AP(
                                    tensor=rs_ap.tensor,
                                    offset=rs_ap.offset,
                                    ap=[list(rs_ap.ap[0]), list(rs_ap.ap[1]), [0, V]],
                                ),
                                op=OP.mult,
                            )
                            Es_ap = E_s[:, 0:un, 0:V]
                            dEs = list(Es_ap.ap)
                            idp_ap = idp_t[:, 0:P]
                            nc.vector.tensor_tensor(
                                diag_s[:, u0 * V : (u0 + un) * V],
                                bass.AP(
                                    tensor=idp_ap.tensor,
                                    offset=idp_ap.offset,
                                    ap=[list(idp_ap.ap[0]), [0, un * V], [1, P]],
                                ),
                                bass.AP(
                                    tensor=Es_ap.tensor,
                                    offset=Es_ap.offset,
                                    ap=[dEs[0], [dEs[2][0], un * V], [0, P]],
                                ),
                                op=OP.mult,
                            )
                        diag_ss.append(diag_s)

                    emit_chain()


                # next group's projections (qT landed half a group ago)
                next_bs = emit_qkqp_mm(qT_next) if qT_next is not None else None

                # previous group's gelu FIRST NOW (emitted here, after this
                # group's exps, so its Gelu<->Exp table loads never precede
                # the exps the reciprocal is waiting on), then its residual
                if pending is not None:
                    p_ps, p_bs, p_g0 = pending
                    p_gl = gwork.tile([P, G, D], BF16, tag="gl")
                    nc.scalar.activation(
                        p_gl, p_ps, AF.Gelu if gelu else AF.Identity
                    )
                    y_out = gwork.tile([P, G, D], BF16, tag="yout")
                    for s2 in range(G // 2):
                        nc.vector.tensor_tensor(
                            y_out[:, 2 * s2 : 2 * s2 + 2],
                            p_gl[:, 2 * s2 : 2 * s2 + 2],
                            p_bs[s2][:, :, D : 2 * D],
                            op=OP.add,
                        )
                    nc.scalar.dma_start(
                        out=out_d.ap()[:, p_g0 : p_g0 + G], in_=y_out
                    )

                for t in range(G):
                    ti = g0 + t
                    v_t = v2_ts[t // 2][:, t % 2]

                    # vmixT[d,n] = sum_w v_w[n,d]*attn_w[n]: stat = v (fp8)
                    vmixT_ps = ps.tile([P, 2, P], F32, tag="vmixT")
                    for h in range(2):
                        for w in range(V):
                            nc.tensor.matmul(
                                vmixT_ps[:, h],
                                v_t[:, w, h * P : (h + 1) * P],
                                diag_ss[t // 2][:, (t % 2) * V + w],
                                start=(w == 0),
                                stop=(w == V - 1),
                            )
                    vT_b = work.tile([P, 2, P], BF16, tag="vT")
                    nc.scalar.copy(vT_b, vmixT_ps)

                    # y = vmixT.T @ MB + bo (bias as a K=1 matmul)
                    nc.tensor.matmul(
                        y_ps[:, t], ones_t, bo_t, start=True, stop=False
                    )
                    nc.tensor.matmul(
                        y_ps[:, t], vT_b[:, 0], mb_t[:, 0], start=False, stop=False
                    )
                    nc.tensor.matmul(
                        y_ps[:, t], vT_b[:, 1], mb_t[:, 1], start=False, stop=True
                    )

                    # LAST group: drain the epilogue per pair so the final
                    # gelu/add/store pipeline with the remaining vmix work
                    # instead of serializing after it (trims the kernel tail)
                    if gi == n_groups - 1 and (t % 2 == 1 or t == G - 2):
                        if t == G - 2 or t == G - 1:
                            lo, n_t = t, 1
                        else:
                            lo, n_t = t - 1, 2
                        gl_h = gwork.tile([P, n_t, D], BF16, tag="glh")
                        nc.scalar.activation(
                            gl_h,
                            y_ps[:, lo : lo + n_t],
                            AF.Gelu if gelu else AF.Identity,
                        )
                        yo_h = gwork.tile([P, n_t, D], BF16, tag="yoh")
                        nc.vector.tensor_tensor(
                            yo_h,
                            gl_h,
                            qkqp_bs[lo // 2][
                                :, lo % 2 : lo % 2 + n_t, D : 2 * D
                            ],
                            op=OP.add,
                        )
                        nc.scalar.dma_start(
                            out=out_d.ap()[:, g0 + lo : g0 + lo + n_t],
                            in_=yo_h,
                        )

                if gi == n_groups - 1:
                    continue
                # gelu deferred: emitted in the next group after its exps
                pending = (y_ps, qkqp_bs, g0)
                cur_bs = next_bs

    nc.compile()
    return nc


_NC_CACHE = {}


def _get_nc(n_tiles: int = N_TILES):
    if n_tiles not in _NC_CACHE:
        _NC_CACHE[n_tiles] = build_bass(n_tiles)
    return _NC_CACHE[n_tiles]


def _host_prep(Wq, Wk, Wv, Wo, bo):
    Wq = np.asarray(Wq, dtype=np.float32)
    Wk = np.asarray(Wk, dtype=np.float32)
    Wv = np.asarray(Wv, dtype=np.float32)
    Wo = np.asarray(Wo, dtype=np.float32)
    bo = np.asarray(bo, dtype=np.float32)
    scale = np.float32(1.0) / np.sqrt(np.float32(D))
    ma = (Wq.T @ Wk) * scale
    mq = Wq.T
    mamq = (
        np.concatenate([ma, mq], axis=1).reshape(2, P, 2 * D).astype(NP_BF16)
    )
    mb = (Wv.T @ Wo.T).reshape(2, P, D).astype(NP_BF16)
    bo_r = bo.reshape(1, D).astype(NP_BF16)
    ones_r = np.ones((1, P), dtype=NP_BF16)
    idp = np.eye(P, dtype=np.float32).astype(NP_BF16)
    return (
        np.ascontiguousarray(mamq),
        np.ascontiguousarray(mb),
        bo_r,
        ones_r,
        idp,
    )


def make_in_maps(q, k, v, Wq, Wk, Wv, Wo, bo):
    q = np.asarray(q, dtype=np.float32)
    k = np.asarray(k, dtype=np.float32)
    v = np.asarray(v, dtype=np.float32)
    mamq, mb, bo_r, ones_r, idp = _host_prep(Wq, Wk, Wv, Wo, bo)
    in_maps = []
    for c in range(N_CORES):
        sl = slice(c * NC_PTS, (c + 1) * NC_PTS)
        # qT: [128(d-half part), 2, npts]
        q_c = np.ascontiguousarray(
            q[0, sl].T.reshape(2, P, NC_PTS).transpose(1, 0, 2)
        ).astype(NP_BF16)
        k_c = k[:, sl].transpose(1, 0, 2).astype(NP_BF16)
        v_c = v[:, sl].transpose(1, 0, 2).astype(NP_FP8)
        in_maps.append(
            {
                "qT": q_c,
                "k": np.ascontiguousarray(k_c),
                "v": np.ascontiguousarray(v_c),
                "mamq": mamq,
                "mb": mb,
                "bo_r": bo_r,
                "ones_r": ones_r,
                "idp": np.ascontiguousarray(idp),
            }
        )
    return in_maps


def gather_out(results):
    """[P, N_TILES, D] bf16 per core -> [8, 32768, 256] f32, 8x replicated."""
    out = np.empty((N_CORES, N_TOTAL, D), dtype=np.float32)
    for c in range(N_CORES):
        y = (
            results[c]["out"]
            .astype(np.float32)
            .transpose(1, 0, 2)
            .reshape(NC_PTS, D)
        )
        out[c] = np.repeat(y, V, axis=0)
    return out


def kernel(q, k, v, Wq, Wk, Wv, Wo, bo):
    nc = _get_nc()
    in_maps = make_in_maps(q, k, v, Wq, Wk, Wv, Wo, bo)
    res = run_bass_kernel_spmd(nc, in_maps, core_ids=list(range(N_CORES)))
    return gather_out(res.results)

